# revision 11
# baseline (speedup 1.0000x reference)
# Trainium2 Bass kernel for nn_BertProber (segment_reduce, memory-bound).
#
# Sharding: pure data parallel over the sentence dim N=1024 -> 8 cores x 128
# sentences. Each core processes its 128 review + 128 reply sentences
# independently; no collectives.
#
# Per-core algorithm (v2: token-skipping gather):
#   Only tokens 0..num_tokens of each sentence contribute to either output
#   (mean nt ~ 65 of 128 rows), so feature loading uses SWDGE dma_gather with
#   per-sentence 128-slot index blocks: slot l holds row 128*n+l when
#   l <= nt[n], else -1. The DGE skips -1 entries (no HBM traffic) while the
#   destination position is fixed by list position, so gathered rows land in
#   the same [token_partition, sentence_slot] tile layout the matmuls consume;
#   skipped slots keep stale-but-finite data and the per-sentence weight
#   vectors are zero there. This cuts feature-read traffic to ~52%.
#
#   1. Weight vectors w_pt/w_sent[n, l] built on DVE from iota constants
#      (span union via difference-of-step-functions), folding 1/count and the
#      has_span fallback; PE-transposes to [l, n]; ACT interleaves into lhsT
#      columns (pt, sent) per sentence.
#   2. Index tiles built on DVE in the DGE's wrapped [16, 8n+q] layout
#      (replicated to all 128 partitions for the 8 Q7 cores): compare an
#      l-iota constant against nt broadcast via a small DRAM roundtrip,
#      combine with an index iota, cast to int16.
#   3. Main loop: one dma_gather per 8-sentence buffer (1024 idxs, fp32 rows
#      of 3KB) into a 2-deep fp32 staging tile; DVE bulk-casts each buffer to
#      bf16 (this walrus rejects float32r matmuls and fp32 streams at 4
#      cyc/row); per sentence two bf16 matmuls (H split 512+256) with 32
#      interleaved lhsT columns per 4-sentence group.
#   4. ACT drains PSUM -> staging; single-partition DMAs scatter pt/sent rows.
#
# Raw bass (manual semaphores); extended instructions (load_library,
# dma_gather) need lower_extended_insts() before compile.
import numpy as np

N, L, H, K = 1024, 128, 768, 5
NCORES = 8
NS = N // NCORES   # sentences per core
G8 = 8             # 4-sentence groups per staging supertile (32 sentences)
NBUF = 4           # feature-tile buffering depth
NGRP = NS // 4     # groups per kind
NSG = NGRP // G8   # supertiles per kind
NIDX = 1024        # indices per gather (8 sentences x 128 slots)

MM_DTYPE = "bfloat16"  # matmul dtype: 1 PE cycle/row

_CACHE = {}


def _build_nc(repeat=1, swap_last=False):
    import concourse.bass as bass
    import concourse.mybir as mybir
    from concourse.library_config import mlp
    from concourse.library_overlay import lower_extended_insts
    from contextlib import ExitStack

    f32 = mybir.dt.float32
    i16 = mybir.dt.int16
    i32 = mybir.dt.int32
    mmdt = getattr(mybir.dt, MM_DTYPE)
    Alu = mybir.AluOpType

    nc = bass.Bass(trn_type="TRN2")

    kinds = ("rv", "rp")
    ins = {}
    outs = {}
    scratch = {}
    for kind in kinds:
        ins[f"{kind}_feat"] = nc.dram_tensor(
            f"{kind}_feat", [NS * L, H], f32, kind="ExternalInput")
        ins[f"{kind}_nt"] = nc.dram_tensor(
            f"{kind}_nt", [NS], i32, kind="ExternalInput")
        ins[f"{kind}_ss"] = nc.dram_tensor(
            f"{kind}_ss", [NS, K], i32, kind="ExternalInput")
        ins[f"{kind}_se"] = nc.dram_tensor(
            f"{kind}_se", [NS, K], i32, kind="ExternalInput")
        outs[f"{kind}_pt"] = nc.dram_tensor(
            f"{kind}_pt", [NS, H], f32, kind="ExternalOutput")
        outs[f"{kind}_sent"] = nc.dram_tensor(
            f"{kind}_sent", [NS, H], f32, kind="ExternalOutput")
        # nt broadcast roundtrip: [16, NS*8] fp32 written row-by-row
        scratch[kind] = nc.dram_tensor(f"ntsc_{kind}", [16, NS * 8], f32)

    # Compile-time constants.
    iota_row = np.tile(np.arange(L, dtype=np.float32)[None, :], (128, 1))
    iota_sent_np = iota_row.copy()
    iota_sent_np[:, 0] = 1000.0  # position 0 ([CLS]) never in the sentence mask
    ident_np = np.eye(128, dtype=np.float32)
    # Index-build constants in the DGE wrapped layout [c, j=8n+q]:
    #   l(c, j) = 16*(j%8) + c%16 ; idx+1 = 128*(j//8) + l + 1
    cc = np.arange(128, dtype=np.float32)[:, None] % 16
    jj = np.arange(NS * 8, dtype=np.float32)[None, :]
    cl16_np = 16.0 * (jj % 8) + cc                      # l value (0..127)
    cip16_np = 128.0 * np.floor(jj / 8) + cl16_np + 1.0  # idx+1
    iota_span_d = nc.inline_tensor(iota_row, name="iota_span_c")
    iota_sent_d = nc.inline_tensor(iota_sent_np, name="iota_sent_c")
    ident_d = nc.inline_tensor(ident_np, name="ident_c")
    cl16_d = nc.inline_tensor(cl16_np, name="cl16_c")
    cip16_d = nc.inline_tensor(cip16_np, name="cip16_c")

    with ExitStack() as ctx:
        def sb(name, shape, dt):
            return ctx.enter_context(nc.sbuf_tensor(name, shape, dt))

        def ps(name, shape, dt):
            return ctx.enter_context(nc.psum_tensor(name, shape, dt))

        def sem(name):
            return ctx.enter_context(nc.semaphore(name))

        iota_span = sb("iota_span", [128, L], f32)
        iota_sent = sb("iota_sent", [128, L], f32)
        ident = sb("ident", [128, 128], f32)
        cl16 = sb("cl16", [128, NS * 8], f32)
        cip16 = sb("cip16", [128, NS * 8], f32)
        tnt16 = {k: sb(f"tnt16_{k}", [128, NS * 8], f32) for k in kinds}
        idx16 = {k: sb(f"idx16_{k}", [128, NS * 8], i16) for k in kinds}
        dense16 = sb("dense16", [128, 128], i16)
        msk16 = sb("msk16", [128, NS * 8], f32)
        idxf = sb("idxf", [128, NS * 8], f32)
        zeros8 = sb("zeros8", [NS, 8], f32)
        nt8 = {k: sb(f"nt8_{k}", [NS, 8], f32) for k in kinds}
        # per-kind weight-phase tiles
        wtiles = {}
        for kind in kinds:
            wtiles[kind] = {
                "nt_i": sb(f"nt_i_{kind}", [NS, 1], i32),
                "ss_i": sb(f"ss_i_{kind}", [NS, K], i32),
                "se_i": sb(f"se_i_{kind}", [NS, K], i32),
                "ntf": sb(f"ntf_{kind}", [NS, 1], f32),
                "ssf": sb(f"ssf_{kind}", [NS, K], f32),
                "sep1": sb(f"sep1_{kind}", [NS, K], i32),
                "sep1f": sb(f"sep1f_{kind}", [NS, K], f32),
                "w_pt": sb(f"w_pt_{kind}", [NS, L], f32),
                "w_sent": sb(f"w_sent_{kind}", [NS, L], f32),
                "wT": sb(f"wT_{kind}", [128, 2 * NS + 32], mmdt),
            }
        # shared DVE scratch
        sm = sb("sm", [NS, L], f32)
        spm = sb("spm", [NS, L], f32)
        tmp = sb("tmp", [NS, L], f32)
        cnt_s = sb("cnt_s", [NS, 1], f32)
        cnt_p = sb("cnt_p", [NS, 1], f32)
        has = sb("has", [NS, 1], f32)
        cntp_c = sb("cntp_c", [NS, 1], f32)
        recip_p = sb("recip_p", [NS, 1], f32)
        recip_s = sb("recip_s", [NS, 1], f32)
        rp_sel = sb("rp_sel", [NS, 1], f32)
        nh = sb("nh", [NS, 1], f32)
        rs_sel = sb("rs_sel", [NS, 1], f32)
        sigt = sb("sigt", [1, 1], f32)

        ft32 = [sb(f"ft32_{i}", [128, 8, H], f32) for i in range(2)]
        ftb = [sb(f"ftb{i}", [128, 8, H], mmdt) for i in range(NBUF)]
        stage = [sb(f"stage{i}", [128, G8, H], f32) for i in range(2)]
        psA = [ps(f"psA{i}", [128, 512], f32) for i in range(2)]
        psB = [ps(f"psB{i}", [128, 512], f32) for i in range(2)]

        const_sem = sem("const_sem")   # iota/ident/cl/cip DMAs   (SP -> DVE/PE)
        wdma_sem = sem("wdma_sem")    # nt/ss/se DMAs            (SP -> DVE)
        nt8_sem = sem("nt8_sem")     # nt8 tiles ready          (DVE -> SP)
        scrw_sem = {k: sem(f"scrw_{k}") for k in kinds}  # scratch writes done
        tntr_sem = {k: sem(f"tntr_{k}") for k in kinds}  # tnt16 reads done
        idx_sem = sem("idx_sem")     # per-kind idx16 ready     (DVE -> Pool)
        dn_sem = sem("dn_sem")      # dense idx ready          (DVE -> Pool)
        cast_sem = sem("cast_sem")    # per-buffer bf16 cast done (DVE -> PE/Pool)
        dve_sem = sem("dve_sem")     # per-kind weights ready   (DVE -> PE)
        pe_w_sem = sem("pe_w_sem")    # per-kind transposes      (PE -> ACT)
        wact_sem = sem("wact_sem")    # per-kind wT interleave   (ACT -> PE)
        ft_sem = [sem(f"ft_sem{i}") for i in range(2)]  # gathers (Pool -> DVE)
        pe_grp = sem("pe_grp")      # per-group matmuls done   (PE -> ACT, Pool)
        act_grp = sem("act_grp")     # per-group drains done    (ACT -> PE)
        odma_sem = [sem(f"odma_sem{i}") for i in range(2)]  # out DMAs per stage
        dve_chain = sem("dve_chain")  # same-engine RAW ordering on DVE
        act_chain = sem("act_chain")  # ACT same-engine ordering

        with nc.Block() as block:

            @block.sync
            def _(sync):
                sync.dma_start(out=iota_span[:], in_=iota_span_d[:, :]).then_inc(const_sem, 16)
                sync.dma_start(out=iota_sent[:], in_=iota_sent_d[:, :]).then_inc(const_sem, 16)
                sync.dma_start(out=ident[:], in_=ident_d[:, :]).then_inc(const_sem, 16)
                sync.dma_start(out=cl16[:], in_=cl16_d[:, :]).then_inc(const_sem, 16)
                sync.dma_start(out=cip16[:], in_=cip16_d[:, :]).then_inc(const_sem, 16)
                for kind in kinds:
                    w = wtiles[kind]
                    sync.dma_start(out=w["nt_i"][:], in_=ins[f"{kind}_nt"][:].unsqueeze(1)).then_inc(wdma_sem, 16)
                    sync.dma_start(out=w["ss_i"][:], in_=ins[f"{kind}_ss"][:, :]).then_inc(wdma_sem, 16)
                    sync.dma_start(out=w["se_i"][:], in_=ins[f"{kind}_se"][:, :]).then_inc(wdma_sem, 16)
                # nt broadcast roundtrip: 16 row writes then 8 replicated reads
                for kidx, kind in enumerate(kinds):
                    sync.wait_ge(nt8_sem, kidx + 1)
                    for c in range(16):
                        sync.dma_start(
                            out=scratch[kind][c:c + 1, :], in_=nt8[kind][:, :]
                        ).then_inc(scrw_sem[kind], 16)
                for kind in kinds:
                    sync.wait_ge(scrw_sem[kind], 256)
                    for r in range(8):
                        sync.dma_start(
                            out=tnt16[kind][16 * r:16 * r + 16, :],
                            in_=scratch[kind][:, :],
                        ).then_inc(tntr_sem[kind], 16)

            @block.gpsimd
            def _(gpsimd):
                gpsimd.load_library(mlp)
                for rep in range(repeat):
                  for kidx, kind in enumerate(kinds):
                    feat = ins[f"{kind}_feat"]
                    if kidx == 1 or rep > 0:
                        gpsimd.wait_ge(idx_sem, kidx + 1)
                    for gb in range(NS // 8):
                        bgg = (rep * 2 + kidx) * (NS // 8) + gb
                        # First fills of ft32[0]/ft32[1] use dense index lists
                        # so no uninitialized SBUF survives (a skipped gather
                        # slot only holds prior FINITE data afterwards).
                        idx_ap = idx16[kind][:, 64 * gb:64 * gb + 64]
                        if bgg < 2:
                            gpsimd.wait_ge(dn_sem, 1)
                            idx_ap = dense16[:, 64 * bgg:64 * bgg + 64]
                        else:
                            if bgg == 2 and kidx == 0 and rep == 0:
                                gpsimd.wait_ge(idx_sem, 1)
                            # previous occupant of this ft32 buffer cast out
                            gpsimd.wait_ge(cast_sem, bgg - 1)
                        gpsimd.dma_gather(
                            ft32[bgg % 2][:],
                            feat[:, :],
                            idx_ap,
                            NIDX, NIDX, H,
                        ).then_inc(ft_sem[bgg % 2], 16)

            @block.vector
            def _(vector):
                # The DVE pipeline has no interlock for back-to-back RAW, so
                # chain every op through a self-semaphore.
                nv = [0]

                def dv(res):
                    res.then_inc(dve_chain, 1)
                    nv[0] += 1

                def dw():
                    if nv[0]:
                        vector.wait_ge(dve_chain, nv[0])

                def sig(sem_):
                    # external signal AFTER the chained producer retired
                    dw()
                    vector.memset(sigt[:], 0.0).then_inc(sem_, 1)

                vector.wait_ge(const_sem, 80)
                # dense index lists for the ft0/ft1 first fills
                dv(vector.tensor_scalar(
                    out=idxf[:, 0:128], in0=cip16[:, 0:128], scalar1=-1.0,
                    scalar2=None, op0=Alu.add))
                dw()
                dv(vector.tensor_copy(out=dense16[:], in_=idxf[:, 0:128]))
                sig(dn_sem)
                vector.wait_ge(wdma_sem, 96)
                # nt -> float, broadcast x8 (for the scratch roundtrip)
                dv(vector.memset(zeros8[:], 0.0))
                for kind in kinds:
                    dv(vector.tensor_copy(out=wtiles[kind]["ntf"][:], in_=wtiles[kind]["nt_i"][:]))
                dw()
                for kidx, kind in enumerate(kinds):
                    dv(vector.tensor_scalar(
                        out=nt8[kind][:], in0=zeros8[:],
                        scalar1=wtiles[kind]["ntf"][:], scalar2=None,
                        op0=Alu.add))
                    sig(nt8_sem)
                # index tiles (gathers are the long pole; do these first)
                for kidx, kind in enumerate(kinds):
                    vector.wait_ge(tntr_sem[kind], 128)
                    dv(vector.tensor_tensor(
                        out=msk16[:], in0=cl16[:], in1=tnt16[kind][:], op=Alu.is_le))
                    dw()
                    dv(vector.tensor_tensor(
                        out=idxf[:], in0=cip16[:], in1=msk16[:], op=Alu.mult))
                    dw()
                    dv(vector.tensor_scalar(
                        out=idxf[:], in0=idxf[:], scalar1=-1.0, scalar2=None,
                        op0=Alu.add))
                    dw()
                    dv(vector.tensor_copy(out=idx16[kind][:], in_=idxf[:]))
                    sig(idx_sem)
                # weight phase
                for kidx, kind in enumerate(kinds):
                    w = wtiles[kind]
                    dv(vector.tensor_copy(out=w["ssf"][:], in_=w["ss_i"][:]))
                    dv(vector.tensor_scalar(
                        out=w["sep1"][:], in0=w["se_i"][:], scalar1=1,
                        scalar2=None, op0=Alu.add))
                    dw()
                    dv(vector.tensor_copy(out=w["sep1f"][:], in_=w["sep1"][:]))
                    # sentence mask + count
                    dw()
                    dv(vector.tensor_scalar(
                        out=sm[:], in0=iota_sent[:], scalar1=w["ntf"][:],
                        scalar2=0.0, op0=Alu.is_le, op1=Alu.add,
                        accum_out=cnt_s[:]))
                    # span union mask: sum_k [ge(l, s_k) - ge(l, e_k + 1)]
                    dv(vector.tensor_scalar(
                        out=spm[:], in0=iota_span[:], scalar1=w["ssf"][:, 0:1],
                        scalar2=None, op0=Alu.is_ge))
                    for k in range(1, K):
                        dw()
                        dv(vector.tensor_scalar(
                            out=tmp[:], in0=iota_span[:],
                            scalar1=w["ssf"][:, k:k + 1],
                            scalar2=None, op0=Alu.is_ge))
                        dw()
                        dv(vector.tensor_tensor(out=spm[:], in0=spm[:], in1=tmp[:], op=Alu.add))
                    for k in range(K):
                        dw()
                        dv(vector.tensor_scalar(
                            out=tmp[:], in0=iota_span[:],
                            scalar1=w["sep1f"][:, k:k + 1],
                            scalar2=None, op0=Alu.is_ge))
                        dw()
                        dv(vector.tensor_tensor(out=spm[:], in0=spm[:], in1=tmp[:], op=Alu.subtract))
                    dw()
                    dv(vector.reduce_sum(out=cnt_p[:], in_=spm[:], axis=mybir.AxisListType.X))
                    dw()
                    dv(vector.tensor_scalar(
                        out=has[:], in0=cnt_p[:], scalar1=1.0, scalar2=None, op0=Alu.is_ge))
                    dv(vector.tensor_scalar(
                        out=cntp_c[:], in0=cnt_p[:], scalar1=1.0, scalar2=None, op0=Alu.max))
                    dw()
                    dv(vector.reciprocal(out=recip_p[:], in_=cntp_c[:]))
                    dv(vector.reciprocal(out=recip_s[:], in_=cnt_s[:]))
                    dw()
                    dv(vector.tensor_tensor(out=rp_sel[:], in0=has[:], in1=recip_p[:], op=Alu.mult))
                    dv(vector.tensor_scalar(
                        out=nh[:], in0=has[:], scalar1=-1.0, scalar2=1.0,
                        op0=Alu.mult, op1=Alu.add))
                    dw()
                    dv(vector.tensor_tensor(out=rs_sel[:], in0=nh[:], in1=recip_s[:], op=Alu.mult))
                    dv(vector.tensor_scalar(
                        out=w["w_sent"][:], in0=sm[:], scalar1=recip_s[:],
                        scalar2=None, op0=Alu.mult))
                    dv(vector.tensor_scalar(
                        out=w["w_pt"][:], in0=spm[:], scalar1=rp_sel[:],
                        scalar2=None, op0=Alu.mult))
                    dw()
                    dv(vector.tensor_scalar(
                        out=tmp[:], in0=sm[:], scalar1=rs_sel[:],
                        scalar2=None, op0=Alu.mult))
                    dw()
                    dv(vector.tensor_tensor(out=w["w_pt"][:], in0=w["w_pt"][:], in1=tmp[:], op=Alu.add))
                    # zero pad columns (so M=32 matmuls can slide past the end)
                    dw()
                    vector.memset(w["wT"][:, 2 * NS:], 0.0).then_inc(dve_sem, 1)
                # bf16 cast loop: one tensor_copy per gathered buffer
                ncast = 0
                for rep in range(repeat):
                  for kidx, kind in enumerate(kinds):
                    for gb in range(NS // 8):
                        bgg = (rep * 2 + kidx) * (NS // 8) + gb
                        vector.wait_ge(ft_sem[bgg % 2], 16 * (bgg // 2 + 1))
                        if bgg >= NBUF:
                            # PE done with this ftb tile's previous occupant
                            vector.wait_ge(pe_grp, 2 * (bgg - NBUF + 1))
                        dv(vector.tensor_copy(
                            out=ftb[bgg % NBUF][:], in_=ft32[bgg % 2][:]))
                        sig(cast_sem)
                        ncast += 1

            @block.tensor
            def _(tensor):
                # weight transposes [n, L] -> [L, n], into main-loop PSUM banks
                for kidx, kind in enumerate(kinds):
                    w = wtiles[kind]
                    tensor.wait_ge(dve_sem, kidx + 1)
                    tensor.transpose(psA[kidx][:, 0:128], w["w_pt"][:], ident[:])
                    tensor.transpose(
                        psB[kidx][:, 0:128], w["w_sent"][:], ident[:]
                    ).then_inc(pe_w_sem, 1)
                # main loop
                tensor.wait_ge(wact_sem, 2)
                for rep in range(repeat):
                  for kidx, kind in enumerate(kinds):
                    wT = wtiles[kind]["wT"]
                    for g in range(NGRP):
                        gg = (rep * 2 + kidx) * NGRP + g
                        pb = gg % 2
                        bgg = gg // 2
                        if gg % 2 == 0:
                            tensor.wait_ge(cast_sem, bgg + 1)
                        if gg >= 2:
                            tensor.wait_ge(act_grp, gg - 1)
                        for j in range(4):
                            n_ = 4 * g + j
                            sl = (g % 2) * 4 + j
                            lhsT = wT[:, 2 * n_:2 * n_ + 32]
                            tensor.matmul(
                                out=psA[pb][32 * j:32 * j + 32, :], lhsT=lhsT,
                                rhs=ftb[bgg % NBUF][:, sl, 0:512],
                                start=True, stop=True, tile_position=(0, 32 * j))
                            mm = tensor.matmul(
                                out=psB[pb][32 * j:32 * j + 32, 0:256], lhsT=lhsT,
                                rhs=ftb[bgg % NBUF][:, sl, 512:H],
                                start=True, stop=True, tile_position=(0, 32 * j))
                        mm.then_inc(pe_grp, 1)

            @block.scalar
            def _(scalar):
                # wT interleave: even cols = pt weights, odd cols = sent weights
                for kidx, kind in enumerate(kinds):
                    w = wtiles[kind]
                    wT_v = w["wT"][:, 0:2 * NS].rearrange("p (n t) -> p t n", t=2)
                    scalar.wait_ge(pe_w_sem, kidx + 1)
                    scalar.copy(out=wT_v[:, 0, :], in_=psA[kidx][:, 0:128])
                    cp = scalar.copy(out=wT_v[:, 1, :], in_=psB[kidx][:, 0:128])
                    cp.then_inc(wact_sem, 1)
                # main loop: drains + output DMAs
                for rep in range(repeat):
                  for kidx, kind in enumerate(kinds):
                    pt_out = outs[f"{kind}_pt"]
                    sent_out = outs[f"{kind}_sent"]
                    for g in range(NGRP):
                        gg = (rep * 2 + kidx) * NGRP + g
                        pb = gg % 2
                        sgg = gg // G8
                        st = stage[sgg % 2]
                        if g % G8 == 0 and sgg >= 2:
                            scalar.wait_ge(odma_sem[sgg % 2], 128 * (sgg // 2))
                        scalar.wait_ge(pe_grp, gg + 1)
                        scalar.copy(out=st[:, g % G8, 0:512], in_=psA[pb][:])
                        cp = scalar.copy(out=st[:, g % G8, 512:H], in_=psB[pb][:, 0:256])
                        cp.then_inc(act_grp, 1)
                        if g % G8 == G8 - 1:
                            # ensure the staging writes retired before the
                            # DMA engines read them
                            scalar.wait_ge(act_grp, gg + 1)
                            base = 4 * G8 * (g // G8)
                            o1, o2 = pt_out, sent_out
                            if swap_last and rep == repeat - 1:
                                o1, o2 = sent_out, pt_out
                            for j in range(4):
                                scalar.dma_start(
                                    out=o1[base + j:base + 4 * G8:4, :].unsqueeze(0),
                                    in_=st[32 * j:32 * j + 1, :, :],
                                ).then_inc(odma_sem[sgg % 2], 16)
                                scalar.dma_start(
                                    out=o2[base + j:base + 4 * G8:4, :].unsqueeze(0),
                                    in_=st[32 * j + 1:32 * j + 2, :, :],
                                ).then_inc(odma_sem[sgg % 2], 16)
                total_sg = 2 * NSG * repeat
                scalar.wait_ge(odma_sem[0], 128 * ((total_sg + 1) // 2))
                scalar.wait_ge(odma_sem[1], 128 * (total_sg // 2))

    lower_extended_insts(nc)
    return nc


def _get_nc():
    if "nc" not in _CACHE:
        _CACHE["nc"] = _build_nc()
    return _CACHE["nc"]


def _make_in_maps(inputs):
    def npa(x, dt):
        return np.ascontiguousarray(np.asarray(x), dtype=dt)

    full = {
        "rv_feat": npa(inputs["review_feat"], np.float32).reshape(N * L, H),
        "rp_feat": npa(inputs["reply_feat"], np.float32).reshape(N * L, H),
        "rv_nt": npa(inputs["review_num_tokens"], np.int32),
        "rp_nt": npa(inputs["reply_num_tokens"], np.int32),
        "rv_ss": npa(inputs["review_span_start"], np.int32),
        "rv_se": npa(inputs["review_span_end"], np.int32),
        "rp_ss": npa(inputs["reply_span_start"], np.int32),
        "rp_se": npa(inputs["reply_span_end"], np.int32),
    }
    in_maps = []
    for c in range(NCORES):
        m = {}
        for k, v in full.items():
            if k.endswith("_feat"):
                m[k] = v[c * NS * L:(c + 1) * NS * L]
            else:
                m[k] = v[c * NS:(c + 1) * NS]
        in_maps.append(m)
    return in_maps


def _gather(results):
    def cat(name):
        return np.concatenate([results[c][name] for c in range(NCORES)], axis=0)

    return cat("rv_pt"), cat("rv_sent"), cat("rp_pt"), cat("rp_sent")


def kernel(**inputs):
    from concourse.bass_utils import run_bass_kernel_spmd

    nc = _get_nc()
    in_maps = _make_in_maps(inputs)
    res = run_bass_kernel_spmd(nc, in_maps, list(range(NCORES)))
    return _gather(res.results)
